# revision 19
# baseline (speedup 1.0000x reference)
"""Multi-head self-attention (B=4, L=2048, D=1024, H=16, RoPE, causal) on 8
Trainium2 NeuronCores.

Sharding: data-parallel over batch (4) x tensor-parallel over head groups (2).
Core i handles batch i//2, heads 8*(i%2) .. 8*(i%2)+8.  Each core computes its
QKV projection slice, RoPE, causal attention for its 8 heads, and a partial
output projection over its 512 d-columns; the host sums the two partials per
batch.

On-core dataflow (per core, all matmul operands bf16, psum/softmax fp32):
  qkT[e,l] = Wqk_sub @ x^T       (e = 8 q-heads then 8 k-heads, dh-major)
  rope on qkT rows (pair-swap via stream_shuffle + cos/sin tables)
  V[l,e]   = x @ Wv_sub^T        (natural orientation, 8 heads * 64)
  per head-pair (2 heads stacked in one 128-partition chunk):
    S^T[k,q] = K^T Q  (row-paired K=64 matmuls, tile_position (0,0)/(64,0))
    P^T = exp(S^T/8) with causal masking (valid-range exp + tri-mask)
    s[q] = colsum(P^T) broadcast via ones-matmul (col-paired (0,0)/(0,64))
    O^T[dh,q] = V^T-matmul accumulation (col-paired)
    O^T /= s
  y[l,e] partial = O^T.T @ Wo_sub  (lhsT = O^T chunks)
"""
import sys
sys.path.insert(0, "/opt/trn_rl_repo")

import numpy as np
import ml_dtypes

B, L, D, H = 4, 2048, 1024, 16
DH = D // H  # 64
THETA = 100000.0
NCORES = 8
BF = ml_dtypes.bfloat16

_built = None


def _rope_tables():
    # [128, L] f32: rows = 2 stacked heads' dh (64 each), identical per head.
    pos = np.arange(L, dtype=np.float32)
    inv_freq = (1.0 / THETA ** (np.arange(0, DH, 2, dtype=np.float32) / DH))
    ang = pos[None, :] * inv_freq[:, None]              # [32, L]
    cos = np.cos(ang)                                    # [32, L]
    sin = np.sin(ang)
    cos2 = np.repeat(cos, 2, axis=0)                     # rows 2p,2p+1 = cos_p
    sin2 = np.empty((DH, L), np.float32)
    sin2[0::2] = -sin
    sin2[1::2] = sin
    return (np.concatenate([cos2, cos2], 0).astype(np.float32),
            np.concatenate([sin2, sin2], 0).astype(np.float32))


def _build():
    import concourse.mybir as mybir
    import concourse.tile as tile
    from concourse import bacc

    FP32 = mybir.dt.float32
    BF16 = mybir.dt.bfloat16
    MUL = mybir.AluOpType.mult
    ADD = mybir.AluOpType.add
    EXP = mybir.ActivationFunctionType.Exp
    LN = mybir.ActivationFunctionType.Ln
    SWAP_MASK = [i ^ 1 for i in range(32)]

    nc = bacc.Bacc(None, target_bir_lowering=False)
    # DRAM parameters (per-core shapes; host prepares layouts)
    xt_d = nc.declare_dram_parameter("xt", [8, 128, L], BF16, False)        # x^T d-chunks
    wqk_d = nc.declare_dram_parameter("wqk", [8, 8, 128, 128], BF16, False)  # [dchunk, echunk, d, e]
    wv_d = nc.declare_dram_parameter("wv", [8, 128, 512], BF16, False)      # [dchunk, d, e_v]
    wo_d = nc.declare_dram_parameter("wo", [4, 2, 128, 512], BF16, False)   # [dchunk, ehalf, d, e]
    cos_d = nc.declare_dram_parameter("cos2", [128, L], FP32, False)
    sin_d = nc.declare_dram_parameter("sin2", [128, L], FP32, False)
    tri_d = nc.declare_dram_parameter("trimask", [128, 128], BF16, False)
    y_d = nc.declare_dram_parameter("y", [L, D], FP32, True)

    with tile.TileContext(nc) as tc:
        import contextlib
        ctx = contextlib.ExitStack()
        with ctx:
            # ---- resident SBUF pools (bufs=1: one slot per tag) ----
            res = ctx.enter_context(tc.tile_pool(name="res", bufs=1))
            # streamed-weight + working pools
            wq_pool = ctx.enter_context(tc.tile_pool(name="wqk", bufs=16))
            rope_pool = ctx.enter_context(tc.tile_pool(name="rope", bufs=3))
            pt_pool = ctx.enter_context(tc.tile_pool(name="pt", bufs=10))
            rec_pool = ctx.enter_context(tc.tile_pool(name="rec", bufs=2))
            y_pool = ctx.enter_context(tc.tile_pool(name="yt", bufs=4))

            xt = [res.tile([128, L], BF16, tag=f"xt{d}", name=f"xt{d}") for d in range(8)]
            qkr = [res.tile([128, L], BF16, tag=f"qkr{c}", name=f"qkr{c}") for c in range(8)]
            vsb = [res.tile([128, 512], BF16, tag=f"v{t}", name=f"v{t}") for t in range(16)]
            wv_sb = [res.tile([128, 512], BF16, tag=f"wv{d}", name=f"wv{d}") for d in range(8)]
            wo_sb = [res.tile([128, 512], BF16, tag=f"wo{i}", name=f"wo{i}") for i in range(8)]
            cos_sb = res.tile([128, L], FP32, tag="cos")
            sin_sb = res.tile([128, L], FP32, tag="sin")
            tri_sb = res.tile([128, 128], BF16, tag="tri")
            ones_sb = res.tile([128, DH], BF16, tag="ones")

            # ---- input DMAs (first chunk weights, then xt halves) ----
            nc.vector.memset(ones_sb, 1.0)

            def load_w(c):
                wts = []
                for d in range(8):
                    w = wq_pool.tile([128, 128], BF16, tag="w", name=f"w_{c}_{d}")
                    nc.sync.dma_start(out=w, in_=wqk_d[d, c])
                    wts.append(w)
                return wts

            def emit_qk_chunk(ps_pool, c, wts=None):
                """QKV projection for qk e-chunk c (128 e-cols) + RoPE."""
                if wts is None:
                    wts = load_w(c)
                for l4 in range(4):
                    lsl = slice(512 * l4, 512 * l4 + 512)
                    qkp = ps_pool.tile([128, 512], FP32, tag="qkps")
                    for d in range(8):
                        nc.tensor.matmul(qkp, wts[d], xt[d][:, lsl],
                                         start=(d == 0), stop=(d == 7))
                    # rope: qkr[c][:,lsl] = qkp*cos + swap(qkp)*sin
                    shf = rope_pool.tile([128, 512], FP32, tag="shf")
                    nc.vector.stream_shuffle(shf, qkp, SWAP_MASK)
                    t1 = rope_pool.tile([128, 512], FP32, tag="t1")
                    nc.vector.tensor_tensor(out=t1, in0=qkp, in1=cos_sb[:, lsl], op=MUL)
                    t2 = rope_pool.tile([128, 512], FP32, tag="t2")
                    nc.vector.tensor_tensor(out=t2, in0=shf, in1=sin_sb[:, lsl], op=MUL)
                    nc.gpsimd.tensor_tensor(out=qkr[c][:, lsl], in0=t1, in1=t2, op=ADD)

            def emit_v_tile(ps_pool, t):
                vp = ps_pool.tile([128, 512], FP32, tag="vps")
                lsl = slice(128 * t, 128 * t + 128)
                for d in range(8):
                    nc.tensor.matmul(vp, xt[d][:, lsl], wv_sb[d],
                                     start=(d == 0), stop=(d == 7))
                nc.scalar.copy(out=vsb[t], in_=vp)

            # prologue DMAs: first two chunks' weights, xt halves, then rest
            w0 = load_w(0)
            w4 = load_w(4)
            for d in range(8):
                nc.sync.dma_start(out=xt[d][:, 0:1024], in_=xt_d[d][:, 0:1024])
            for d in range(8):
                nc.sync.dma_start(out=xt[d][:, 1024:2048], in_=xt_d[d][:, 1024:2048])
            for d in range(8):
                nc.sync.dma_start(out=wv_sb[d], in_=wv_d[d])
            nc.sync.dma_start(out=cos_sb, in_=cos_d[:, :])
            nc.sync.dma_start(out=sin_sb, in_=sin_d[:, :])
            nc.sync.dma_start(out=tri_sb, in_=tri_d[:, :])

            # ---- phases: QKV/V (+early attention S^T/exp) then attention ----
            # PSUM: st(4 banks, fresh) opens first and spans both phases so
            # the first two attention rounds' S^T+exp interleave with the
            # tail QKV chunks; qk(2)+v(2) close before av/s/y open (LIFO).
            ot = [res.tile([128, L], BF16, tag=f"ot{p}", name=f"ot{p}") for p in range(4)]
            deferred = []     # (round_ctx, [(k, pt, vs), ...]) for early rounds

            def emit_st_exp(jq, p, nk, st_ps, out_list):
                """S^T + exp + mask for all ktiles of round (jq,p)."""
                qb0 = 512 * jq
                qt, kt = qkr[p], qkr[4 + p]
                for k in range(nk):
                    kpos = 128 * k
                    vs = max(0, kpos - qb0)
                    st = st_ps.tile([128, 1024], FP32, tag="st")
                    ksl = slice(kpos, kpos + 128)
                    qsl = slice(qb0 + vs, qb0 + 512)
                    nc.tensor.matmul(st[:, vs:512], kt[0:64, ksl],
                                     qt[0:64, qsl], start=True, stop=True,
                                     tile_position=(0, 0))
                    nc.tensor.matmul(st[:, 512 + vs:1024], kt[64:128, ksl],
                                     qt[64:128, qsl], start=True, stop=True,
                                     tile_position=(64, 0))
                    pt = pt_pool.tile([128, 1024], BF16, tag="pt")
                    nc.scalar.activation(out=pt[:, vs:1024], in_=st[:, vs:1024],
                                         func=EXP, scale=0.125)
                    if kpos >= qb0:
                        dsl = slice(vs, vs + 128)
                        dslb = slice(512 + vs, 512 + vs + 128)
                        nc.vector.tensor_tensor(out=pt[:, dsl], in0=pt[:, dsl],
                                                in1=tri_sb, op=MUL)
                        nc.vector.tensor_tensor(out=pt[:, dslb], in0=pt[:, dslb],
                                                in1=tri_sb, op=MUL)
                    out_list.append((k, pt, vs))

            with tc.tile_pool(name="ps_st", bufs=2, space="PSUM") as st_ps:
                with tc.tile_pool(name="ps_qk", bufs=2, space="PSUM") as qk_ps, \
                     tc.tile_pool(name="ps_v", bufs=2, space="PSUM") as v_ps:
                    emit_qk_chunk(qk_ps, 0, w0)
                    emit_qk_chunk(qk_ps, 4, w4)
                    for t in range(16):
                        emit_v_tile(v_ps, t)
                    for dc in range(4):
                        for eh in range(2):
                            nc.sync.dma_start(out=wo_sb[dc * 2 + eh],
                                              in_=wo_d[dc, eh])
                    emit_qk_chunk(qk_ps, 1)
                    emit_qk_chunk(qk_ps, 5)
                    lst = []
                    emit_st_exp(0, 0, 4, st_ps, lst)
                    deferred.append(((0, 0, 4), lst))
                    emit_qk_chunk(qk_ps, 2)
                    emit_qk_chunk(qk_ps, 6)
                    lst = []
                    emit_st_exp(0, 1, 4, st_ps, lst)
                    deferred.append(((0, 1, 4), lst))
                    emit_qk_chunk(qk_ps, 3)
                    emit_qk_chunk(qk_ps, 7)

                with tc.tile_pool(name="ps_av", bufs=1, space="PSUM") as av_ps, \
                     tc.tile_pool(name="ps_s", bufs=1, space="PSUM") as s_ps, \
                     tc.tile_pool(name="ps_y", bufs=2, space="PSUM") as y_ps:
                    pending_norm = [None]
                    proj_queue = []

                    def emit_norm(p, qb0, av, s, n):
                        scr = rec_pool.tile([128, 512], FP32, tag="lns",
                                            name=f"lns{n}")
                        rs = rec_pool.tile([128, 512], FP32, tag="rs",
                                           name=f"rs{n}")
                        nc.vector.reciprocal_approx_accurate(out=rs, in_=s,
                                                             scratch=scr)
                        nc.vector.tensor_tensor(out=ot[p][:, qb0:qb0 + 512],
                                                in0=av, in1=rs, op=MUL)

                    def emit_proj_half(t, eh):
                        lsl = slice(128 * t, 128 * t + 128)
                        yp = y_ps.tile([128, 512], FP32, tag="yps")
                        for dc in range(4):
                            nc.tensor.matmul(yp, ot[dc][:, lsl],
                                             wo_sb[dc * 2 + eh],
                                             start=(dc == 0), stop=(dc == 3))
                        yt = y_pool.tile([128, 512], FP32, tag="yt")
                        nc.vector.tensor_copy(out=yt, in_=yp)
                        nc.sync.dma_start(
                            out=y_d[lsl, 512 * eh:512 * eh + 512], in_=yt)

                    def make_sav(p, jq, nk, av, s):
                        qb0 = 512 * jq

                        def emit_sav(k, pt, vs):
                            first, last = (k == 0), (k == nk - 1)
                            isl = slice(vs, 512)
                            bsl = slice(512 + vs, 1024)
                            vca = 128 * p
                            vcb = 128 * p + 64
                            nc.tensor.matmul(s[0:64, isl], ones_sb, pt[:, isl],
                                             start=first, stop=last,
                                             tile_position=(0, 0),
                                             skip_group_check=True)
                            nc.tensor.matmul(s[64:128, isl], ones_sb, pt[:, bsl],
                                             start=first, stop=last,
                                             tile_position=(0, 64),
                                             skip_group_check=True)
                            nc.tensor.matmul(av[0:64, isl],
                                             vsb[k][:, vca:vca + 64], pt[:, isl],
                                             start=first, stop=last,
                                             tile_position=(0, 0),
                                             skip_group_check=True)
                            nc.tensor.matmul(av[64:128, isl],
                                             vsb[k][:, vcb:vcb + 64], pt[:, bsl],
                                             start=first, stop=last,
                                             tile_position=(0, 64),
                                             skip_group_check=True)
                        return emit_sav

                    # flush the two deferred early rounds
                    for (jq, p, nk), lst in deferred:
                        av = av_ps.tile([128, 512], FP32, tag="av",
                                        name=f"av_d{p}")
                        s = s_ps.tile([128, 512], FP32, tag="s", name=f"s_d{p}")
                        sav = make_sav(p, jq, nk, av, s)
                        for args in lst:
                            sav(*args)
                        emit_norm(p, 512 * jq, av, s, f"d{p}")

                    rounds = [(jq, p) for jq in range(4) for p in range(4)]
                    rounds = [r for r in rounds if r not in ((0, 0), (0, 1))]
                    done_groups = set()
                    for n, (jq, p) in enumerate(rounds):
                        qb0 = 512 * jq
                        nk = 4 * (jq + 1)
                        av = av_ps.tile([128, 512], FP32, tag="av", name=f"av{n}")
                        s = s_ps.tile([128, 512], FP32, tag="s", name=f"s{n}")
                        sav = make_sav(p, jq, nk, av, s)
                        qt, kt = qkr[p], qkr[4 + p]
                        SKEW = 2
                        pending = []
                        for k in range(nk):
                            kpos = 128 * k
                            vs = max(0, kpos - qb0)
                            st = st_ps.tile([128, 1024], FP32, tag="st")
                            ksl = slice(kpos, kpos + 128)
                            qsl = slice(qb0 + vs, qb0 + 512)
                            nc.tensor.matmul(st[:, vs:512], kt[0:64, ksl],
                                             qt[0:64, qsl], start=True, stop=True,
                                             tile_position=(0, 0))
                            nc.tensor.matmul(st[:, 512 + vs:1024], kt[64:128, ksl],
                                             qt[64:128, qsl], start=True, stop=True,
                                             tile_position=(64, 0))
                            pt = pt_pool.tile([128, 1024], BF16, tag="pt")
                            nc.scalar.activation(out=pt[:, vs:1024],
                                                 in_=st[:, vs:1024],
                                                 func=EXP, scale=0.125)
                            if kpos >= qb0:
                                dsl = slice(vs, vs + 128)
                                dslb = slice(512 + vs, 512 + vs + 128)
                                nc.vector.tensor_tensor(out=pt[:, dsl],
                                                        in0=pt[:, dsl],
                                                        in1=tri_sb, op=MUL)
                                nc.vector.tensor_tensor(out=pt[:, dslb],
                                                        in0=pt[:, dslb],
                                                        in1=tri_sb, op=MUL)
                            if k == 1 and pending_norm[0] is not None:
                                pending_norm[0]()
                                pending_norm[0] = None
                            if proj_queue:
                                emit_proj_half(*proj_queue.pop(0))
                            pending.append((k, pt, vs))
                            if len(pending) > SKEW:
                                sav(*pending.pop(0))
                        for args in pending:
                            sav(*args)
                        pending_norm[0] = (lambda p=p, qb0=qb0, av=av, s=s, n=n:
                                           emit_norm(p, qb0, av, s, n))
                        if p == 3:
                            pending_norm[0]()
                            pending_norm[0] = None
                            for t in range(4 * jq, 4 * jq + 4):
                                for eh in range(2):
                                    proj_queue.append((t, eh))
                    for t, eh in proj_queue:
                        emit_proj_half(t, eh)
    nc.compile()
    return nc


def _get_nc():
    global _built
    if _built is None:
        _built = _build()
    return _built


def _in_maps(x, W, Wo):
    x = np.asarray(x, np.float32)
    W = np.asarray(W, np.float32)
    Wo = np.asarray(Wo, np.float32)

    cos2, sin2 = _rope_tables()
    tri = np.zeros((128, 128), np.float32)
    p_idx = np.arange(128)
    tri[p_idx[:, None] <= p_idx[None, :]] = 1.0  # valid: k <= q
    tri = tri.astype(BF)

    in_maps = []
    for core in range(NCORES):
        b, g = core // 2, core % 2
        xt = np.ascontiguousarray(x[b].T).astype(BF)                # [D, L]
        xt = xt.reshape(8, 128, L)
        wq = W[512 * g:512 * g + 512]                                # [512, D]
        wk = W[D + 512 * g:D + 512 * g + 512]
        wv = W[2 * D + 512 * g:2 * D + 512 * g + 512]
        wqk_t = np.ascontiguousarray(
            np.concatenate([wq, wk], 0).T).astype(BF)                # [D, 1024]
        # -> [dchunk, echunk, 128, 128]
        wqk_t = wqk_t.reshape(8, 128, 8, 128).transpose(0, 2, 1, 3)
        wqk_t = np.ascontiguousarray(wqk_t)
        wv_t = np.ascontiguousarray(wv.T).astype(BF).reshape(8, 128, 512)
        wo_t = np.ascontiguousarray(Wo[:, 512 * g:512 * g + 512].T).astype(BF)  # [512, D]
        wo_t = wo_t.reshape(4, 128, 2, 512).transpose(0, 2, 1, 3)
        wo_t = np.ascontiguousarray(wo_t)
        in_maps.append({
            "xt": xt, "wqk": wqk_t, "wv": wv_t, "wo": wo_t,
            "cos2": cos2, "sin2": sin2, "trimask": tri,
        })
    return in_maps


def kernel(x, W, Wo):
    from concourse.bass_utils import run_bass_kernel_spmd

    res = run_bass_kernel_spmd(_get_nc(), _in_maps(x, W, Wo),
                               list(range(NCORES)))
    out = np.empty((B, L, D), np.float32)
    for b in range(B):
        out[b] = res.results[2 * b]["y"] + res.results[2 * b + 1]["y"]
    return out


def _install_ntff_hook_shim():
    """The trimmed repo lacks antenv.axon_hooks; reconstruct it so
    run_bass_kernel_spmd(trace=True) can NTFF-profile through axon."""
    import sys as _sys, types
    if "antenv.axon_hooks" in _sys.modules:
        return
    import antenv  # noqa: F401
    from trn_agent_boot.trn_boot import _ntff_profile_via_ctypes
    hook = _ntff_profile_via_ctypes("/opt/axon/libaxon_pjrt.so")
    mod = types.ModuleType("antenv.axon_hooks")
    mod.set_axon_ntff_profile_hook = lambda h: None
    mod.get_axon_ntff_profile_hook = lambda: hook
    _sys.modules["antenv.axon_hooks"] = mod


def kernel_traced(x, W, Wo, tmpdir=None):
    """Run with NTFF tracing; returns BassKernelResults (trace in tmpdir)."""
    from concourse.bass_utils import run_bass_kernel_spmd

    _install_ntff_hook_shim()
    res = run_bass_kernel_spmd(_get_nc(), _in_maps(x, W, Wo),
                               list(range(NCORES)), trace=True, tmpdir=tmpdir)
    return res.exec_time_ns
